# revision 1
# baseline (speedup 1.0000x reference)
"""Trainium2 Bass kernel for nn_DynamicFiltering.

Computation (per batch b):
  xf = frames of x                     (t, c, h, w)
  y  = LeakyReLU(conv2d(xf, w1, b1), 0.2)
  ker = conv2d(y, w2, b2)              (t, 9, h, w)
  ker = ker - mean_k(ker) + 1/45       (per-pixel kernel over K = t*3*3 = 45)
  out[c,h,w] = sum_{t,k1,k2} x_edge[c,t,h+k1-1,w+k2-1] * ker[t,k1,k2][h,w]

Sharding: 8 cores = 2 batches x 4 H-slabs of 32 rows. Each core gets
pre-padded slabs (host bakes zero padding for convs, edge padding for the
filter patches) so the device program is uniform across cores.

Per-core device program:
  - conv1/conv2 as 9 shifted-offset matmuls accumulating in PSUM (fp32r)
  - LeakyReLU as y0 + (2/3)|y0| with the 0.6 scale folded into w2 host-side
  - per-frame PE transposes bring ker into pixel-partition layout kt
  - kernel normalization + W-edge folds on DVE
  - dynamic filtering with scalar_tensor_tensor (per-partition scalar =
    per-pixel kernel value); the dj column shift is handled by three
    output accumulators plus partition-shifted kt copies (made by DMA,
    which is exempt from the engine start-partition restriction)
  - outputs transposed back via PE; the dj shift collapses to free-dim
    offsets during the merge; DMA out
"""

import numpy as np

DIM = 64
T = 5
H = 128
W = 128
SLAB = 32          # output rows per core
NCORES = 8
GH = 36            # conv grid rows: slab + 2*2 halo
GW = 130           # conv grid cols: W + 2
FR = 34            # filter rows: slab + 2 halo

_PROGRAM_CACHE = {}


def _build_program():
    import concourse.bacc as bacc
    import concourse.mybir as mybir
    from concourse.tile import TileContext

    f32 = mybir.dt.float32
    f32r = mybir.dt.float32r
    Act = mybir.ActivationFunctionType
    Alu = mybir.AluOpType

    nc = bacc.Bacc("TRN2", debug=False)

    xc_d = nc.dram_tensor("xc", [DIM, T, GH, GW], f32r, kind="ExternalInput").ap()
    xt_d = nc.dram_tensor("xt", [W, T, FR, DIM], f32, kind="ExternalInput").ap()
    w1t_d = nc.dram_tensor("w1t", [DIM, 9, DIM], f32r, kind="ExternalInput").ap()
    w2t_d = nc.dram_tensor("w2t", [DIM, 9, 9], f32r, kind="ExternalInput").ap()
    b1_d = nc.dram_tensor("b1c", [DIM, 1], f32, kind="ExternalInput").ap()
    b1s_d = nc.dram_tensor("b1s", [DIM, 1], f32, kind="ExternalInput").ap()
    b2_d = nc.dram_tensor("b2c", [9, 1], f32, kind="ExternalInput").ap()
    ym_d = nc.dram_tensor("ymask", [DIM, 2], f32, kind="ExternalInput").ap()
    em_d = nc.dram_tensor("emask", [W, 1], f32, kind="ExternalInput").ap()
    ef_d = nc.dram_tensor("efold", [W, 1], f32, kind="ExternalInput").ap()
    ea_d = nc.dram_tensor("emA", [W, 1], f32, kind="ExternalInput").ap()
    eb_d = nc.dram_tensor("emB", [W, 1], f32, kind="ExternalInput").ap()
    id_d = nc.dram_tensor("ident", [128, 128], f32, kind="ExternalInput").ap()
    out_d = nc.dram_tensor("out", [DIM, SLAB, W], f32, kind="ExternalOutput").ap()

    with TileContext(nc) as tc:
        with (
            tc.tile_pool(name="consts", bufs=1) as cpool,
            tc.tile_pool(name="xcp", bufs=2) as xcp,
            tc.tile_pool(name="yp", bufs=2) as yp,
            tc.tile_pool(name="stage", bufs=2) as stp,
            tc.tile_pool(name="kerp", bufs=1) as kerp,
            tc.tile_pool(name="ktp", bufs=1) as ktp,
            tc.tile_pool(name="accp", bufs=1) as accp,
            tc.tile_pool(name="obp", bufs=3) as obp,
        ):
            w1t_sb = cpool.tile([DIM, 9, DIM], f32r)
            nc.sync.dma_start(out=w1t_sb, in_=w1t_d)
            w2t_sb = cpool.tile([DIM, 9, 9], f32r)
            nc.sync.dma_start(out=w2t_sb, in_=w2t_d)
            b1_sb = cpool.tile([DIM, 1], f32)
            nc.sync.dma_start(out=b1_sb, in_=b1_d)
            b1s_sb = cpool.tile([DIM, 1], f32)
            nc.sync.dma_start(out=b1s_sb, in_=b1s_d)
            b2_sb = cpool.tile([9, 1], f32)
            nc.sync.dma_start(out=b2_sb, in_=b2_d)
            ym_sb = cpool.tile([DIM, 2], f32)
            nc.sync.dma_start(out=ym_sb, in_=ym_d)
            em_sb = cpool.tile([W, 1], f32)
            nc.sync.dma_start(out=em_sb, in_=em_d)
            id_sb = cpool.tile([128, 128], f32)
            nc.sync.dma_start(out=id_sb, in_=id_d)
            ef_sb = cpool.tile([W, 1], f32)
            nc.sync.dma_start(out=ef_sb, in_=ef_d)
            ea_sb = cpool.tile([W, 1], f32)
            nc.sync.dma_start(out=ea_sb, in_=ea_d)
            eb_sb = cpool.tile([W, 1], f32)
            nc.sync.dma_start(out=eb_sb, in_=eb_d)

            kt = ktp.tile([W, T, SLAB, 9], f32)
            ktr = kt.rearrange("p t r (di dj) -> p t r di dj", di=3, dj=3)
            # partition-shifted kernel copies (DMA is exempt from the engine
            # start-partition restriction): kt_p1[q] = kt[q+1], kt_m1[q] = kt[q-1]
            kt_p1 = ktp.tile([W, T, SLAB, 9], f32)
            kt_m1 = ktp.tile([W, T, SLAB, 9], f32)
            nc.vector.memset(kt_p1[96:128], 0.0)
            nc.vector.memset(kt_m1[0:32], 0.0)
            # three dj-separated accumulators:
            #   acc_dj[q, r, c] += xt[q, t, r+di, c] * m_(t,di,dj)[q - dj + 1, r]
            accs = []
            for dj in range(3):
                a = accp.tile([W, SLAB, DIM], f32, name=f"acc{dj}")
                nc.vector.memset(a, 0.0)
                accs.append(a)
            ksrc = [kt_p1, kt, kt_m1]
            u_sb = accp.tile([W, FR, DIM], f32)

            with (
                tc.tile_pool(name="ps1", bufs=3, space="PSUM") as ps1p,
                tc.tile_pool(name="ps2", bufs=3, space="PSUM") as ps2p,
                tc.tile_pool(name="pst", bufs=2, space="PSUM") as pstp,
            ):
                for f in range(T):
                    xt_f = xcp.tile([W, FR, DIM], f32, tag="xt")
                    nc.sync.dma_start(out=xt_f, in_=xt_d[:, f])
                    if f == 0:
                        nc.gpsimd.tensor_copy(u_sb, xt_f)
                    else:
                        nc.gpsimd.tensor_tensor(u_sb, u_sb, xt_f, Alu.add)
                    xc_f = xcp.tile([DIM, GH, GW], f32r, tag="xc")
                    nc.sync.dma_start(out=xc_f, in_=xc_d[:, f])
                    y_f = yp.tile([DIM, GH, GW], f32r, tag="y")
                    # zero-pad columns read by conv2 (memset can't take f32r)
                    u32 = mybir.dt.uint32
                    nc.gpsimd.memset(y_f[:, 1:35, 0:1].bitcast(u32), 0)
                    nc.gpsimd.memset(y_f[:, 1:35, 129:130].bitcast(u32), 0)

                    # conv1 + leaky relu (scaled by 0.6; compensated in w2t)
                    for rc in range(9):
                        g0 = 1 + 4 * rc
                        nr = 4 if rc < 8 else 2
                        ps = ps1p.tile([DIM, 4, W], f32, tag="ps1")
                        for idx in range(9):
                            di, dj = divmod(idx, 3)
                            rhs = xc_f[:, g0 + di - 1:g0 + di - 1 + nr, dj:dj + W]
                            nc.tensor.matmul(
                                ps[:, :nr, :],
                                lhsT=w1t_sb[:, idx, :],
                                rhs=rhs,
                                start=(idx == 0),
                                stop=(idx == 8),
                            )
                        y0 = stp.tile([DIM, 4, W], f32, tag="y0")
                        a0 = stp.tile([DIM, 4, W], f32, tag="a0")
                        nc.scalar.activation(y0[:, :nr], ps[:, :nr], Act.Identity,
                                             bias=b1_sb, scale=1.0)
                        # a0 = (2/3)|y0| via Abs((2/3) ps + (2/3) b1)
                        nc.scalar.activation(a0[:, :nr], ps[:, :nr], Act.Abs,
                                             bias=b1s_sb, scale=2.0 / 3.0)
                        # y_f = y0 + (2/3)|y0|  == (0.6*y0 + 0.4*|y0|) / 0.6
                        nc.gpsimd.tensor_tensor(
                            y_f[:, g0:g0 + nr, 1:129],
                            y0[:, :nr], a0[:, :nr], Alu.add)

                    # conv2 zero-pads rows outside the image: kill y halo rows
                    # that fall outside (mask is 0 there for edge slabs)
                    nc.scalar.activation(y_f[:, 1:2, 1:129], y_f[:, 1:2, 1:129],
                                          Act.Copy, scale=ym_sb[:, 0:1])
                    nc.scalar.activation(y_f[:, 34:35, 1:129], y_f[:, 34:35, 1:129],
                                          Act.Copy, scale=ym_sb[:, 1:2])

                    # conv2 -> ker_f (9, slab, W); grid row = 2 + r
                    ker_f = kerp.tile([9, SLAB, W], f32, tag="kerf")
                    for rc in range(8):
                        g0 = 2 + 4 * rc
                        ps2 = ps2p.tile([9, 4, W], f32, tag="ps2")
                        for idx in range(9):
                            di, dj = divmod(idx, 3)
                            rhs = y_f[:, g0 + di - 1:g0 + di + 3, dj:dj + W]
                            nc.tensor.matmul(
                                ps2,
                                lhsT=w2t_sb[:, idx, :],
                                rhs=rhs,
                                start=(idx == 0),
                                stop=(idx == 8),
                            )
                        nc.scalar.activation(ker_f[:, 4 * rc:4 * rc + 4, :],
                                             ps2, Act.Identity, bias=b2_sb, scale=1.0)

                    # transpose ker_f (9, r, pc) -> kt[pc, r, 9f..9f+9]
                    for r in range(SLAB):
                        pst = pstp.tile([W, 9], f32, tag="pst")
                        nc.tensor.transpose(pst, ker_f[:, r, :], id_sb[:9, :9])
                        nc.scalar.copy(kt[:, f, r, :], pst)

                    # fold W-edge replicate-pad terms into the dj=1 slot
                    # (raw kernel; the mean correction below compensates):
                    #   pc=0:   m[dj=1] += m[dj=0]   (x col -1 == col 0)
                    #   pc=127: m[dj=1] += m[dj=2]   (x col 128 == col 127)
                    nc.vector.tensor_tensor(ktr[0:1, f, :, :, 1],
                                            ktr[0:1, f, :, :, 1],
                                            ktr[0:1, f, :, :, 0], Alu.add)
                    nc.vector.scalar_tensor_tensor(
                        out=ktr[96:128, f, :, :, 1],
                        in0=ktr[96:128, f, :, :, 2], scalar=em_sb[96:128, :],
                        in1=ktr[96:128, f, :, :, 1], op0=Alu.mult, op1=Alu.add)

                    # shifted copies of this frame's kernel columns
                    nc.sync.dma_start(out=kt_p1[0:127, f], in_=kt[1:128, f])
                    nc.sync.dma_start(out=kt_m1[1:128, f], in_=kt[0:127, f])

                    # un-normalized filtering for this frame (normalization is
                    # unfolded into the c*S term after the loop)
                    for di in range(3):
                        for dj in range(3):
                            kb = ksrc[dj][:, f, :, 3 * di + dj].unsqueeze(2)\
                                .broadcast_to((W, SLAB, DIM))
                            prod = stp.tile([W, SLAB, DIM], f32, tag="prod")
                            nc.vector.tensor_tensor(
                                prod, xt_f[:, di:di + SLAB, :], kb, Alu.mult)
                            nc.vector.tensor_tensor(accs[dj], accs[dj], prod,
                                                    Alu.add)

            # normalization term: out += c * (sum of all 45 patches), with
            # c = 1/45 - mean(ker).  sum45 reads the folded kernel, so undo
            # the fold's double-count at the edge partitions.
            sum45 = ktp.tile([W, SLAB], f32)
            kt_rtn = kt.rearrange("p t r n -> p r t n")
            nc.vector.tensor_reduce(sum45, kt_rtn, axis=mybir.AxisListType.XY,
                                    op=Alu.add)
            c_sb = ktp.tile([W, SLAB], f32)
            nc.vector.tensor_scalar(c_sb, sum45, -1.0 / 45.0, 1.0 / 45.0,
                                    Alu.mult, Alu.add)
            corr = ktp.tile([W, SLAB], f32)
            ktr_r = kt.rearrange("p t r (di dj) -> p r t di dj", di=3, dj=3)
            nc.vector.tensor_reduce(corr[0:32], ktr_r[0:32, :, :, :, 0],
                                    axis=mybir.AxisListType.XY, op=Alu.add)
            nc.vector.tensor_reduce(corr[96:128], ktr_r[96:128, :, :, :, 2],
                                    axis=mybir.AxisListType.XY, op=Alu.add)
            nc.vector.scalar_tensor_tensor(out=c_sb[0:32], in0=corr[0:32],
                                           scalar=ea_sb[0:32], in1=c_sb[0:32],
                                           op0=Alu.mult, op1=Alu.add)
            nc.vector.scalar_tensor_tensor(out=c_sb[96:128], in0=corr[96:128],
                                           scalar=eb_sb[96:128], in1=c_sb[96:128],
                                           op0=Alu.mult, op1=Alu.add)

            # S = 3-row vertical box of U (edge rows already clamped in xt)
            s_sb = accp.tile([W, SLAB, DIM], f32)
            nc.vector.tensor_tensor(s_sb, u_sb[:, 0:SLAB, :],
                                    u_sb[:, 1:SLAB + 1, :], Alu.add)
            nc.vector.tensor_tensor(s_sb, s_sb, u_sb[:, 2:SLAB + 2, :], Alu.add)

            # shifted + edge-doubled variants of c
            c_p1 = ktp.tile([W, SLAB], f32)
            c_m1 = ktp.tile([W, SLAB], f32)
            nc.vector.memset(c_p1[96:128], 0.0)
            nc.vector.memset(c_m1[0:32], 0.0)
            nc.sync.dma_start(out=c_p1[0:127], in_=c_sb[1:128])
            nc.sync.dma_start(out=c_m1[1:128], in_=c_sb[0:127])
            c_c = ktp.tile([W, SLAB], f32)
            nc.vector.tensor_scalar(c_c, c_sb, ef_sb, None, Alu.mult)
            for dj, csrc in ((0, c_p1), (1, c_c), (2, c_m1)):
                cb = csrc.unsqueeze(2).broadcast_to((W, SLAB, DIM))
                prod = stp.tile([W, SLAB, DIM], f32, tag="prod")
                nc.vector.tensor_tensor(prod, s_sb, cb, Alu.mult)
                nc.vector.tensor_tensor(accs[dj], accs[dj], prod, Alu.add)

            # transpose accs (q, r, c) -> (r c, q) chunks; the dj shift is a
            # free-dim offset after transposition:
            #   out[m, pc] = T(acc1)[m, pc] + T(acc0)[m, pc-1] + T(acc2)[m, pc+1]
            a0f = accs[0].rearrange("p r c -> p (r c)")
            a1f = accs[1].rearrange("p r c -> p (r c)")
            a2f = accs[2].rearrange("p r c -> p (r c)")
            out_rcw = out_d.rearrange("c r w -> r c w")
            with tc.tile_pool(name="pso", bufs=2, space="PSUM") as psop:
                for oc in range(16):
                    sl = slice(128 * oc, 128 * (oc + 1))
                    p0 = psop.tile([128, 128], f32, tag="pso0")
                    p1 = psop.tile([128, 128], f32, tag="pso1")
                    p2 = psop.tile([128, 128], f32, tag="pso2")
                    nc.tensor.transpose(p0, a0f[:, sl], id_sb)
                    nc.tensor.transpose(p1, a1f[:, sl], id_sb)
                    nc.tensor.transpose(p2, a2f[:, sl], id_sb)
                    ob = obp.tile([128, 128], f32, tag="ob")
                    nc.vector.tensor_copy(ob, p1)
                    nc.vector.tensor_tensor(ob[:, 1:128], ob[:, 1:128],
                                            p0[:, 0:127], Alu.add)
                    nc.vector.tensor_tensor(ob[:, 0:127], ob[:, 0:127],
                                            p2[:, 1:128], Alu.add)
                    nc.sync.dma_start(out=out_rcw[2 * oc:2 * oc + 2], in_=ob)

    return nc


def _get_program():
    if "nc" not in _PROGRAM_CACHE:
        nc = _build_program()
        nc.finalize()
        _PROGRAM_CACHE["nc"] = nc
    return _PROGRAM_CACHE["nc"]


def _host_prep(x, w1, b1, w2, b2):
    """Build the 8 per-core input maps from full inputs."""
    x = np.asarray(x, dtype=np.float32)
    w1 = np.asarray(w1, dtype=np.float32)
    b1 = np.asarray(b1, dtype=np.float32)
    w2 = np.asarray(w2, dtype=np.float32)
    b2 = np.asarray(b2, dtype=np.float32)

    # w1t[ci, 3*di+dj, o] = w1[o, ci, di, dj]
    w1t = np.ascontiguousarray(w1.transpose(1, 2, 3, 0).reshape(DIM, 9, DIM))
    # w2t[ci, 3*di+dj, o] = 0.6 * w2[o, ci, di, dj]   (leaky-relu scale fold)
    w2t = np.ascontiguousarray(0.6 * w2.transpose(1, 2, 3, 0).reshape(DIM, 9, 9))
    b1c = np.ascontiguousarray(b1.reshape(DIM, 1))
    b1s = np.ascontiguousarray((2.0 / 3.0) * b1.reshape(DIM, 1))
    b2c = np.ascontiguousarray(b2.reshape(9, 1))
    ident = np.eye(128, dtype=np.float32)
    emask = np.zeros((W, 1), dtype=np.float32)
    emask[127, 0] = 1.0
    efold = np.ones((W, 1), dtype=np.float32)
    efold[0, 0] = 2.0
    efold[127, 0] = 2.0
    emA = np.zeros((W, 1), dtype=np.float32)
    emA[0, 0] = 1.0 / 45.0
    emB = np.zeros((W, 1), dtype=np.float32)
    emB[127, 0] = 1.0 / 45.0

    in_maps = []
    for core in range(NCORES):
        b, s = divmod(core, 4)
        r0 = s * SLAB
        # conv input: rows r0-2 .. r0+33 zero padded, cols -1..128 zero padded
        xc = np.zeros((DIM, T, GH, GW), dtype=np.float32)
        lo = max(0, r0 - 2)
        hi = min(H, r0 + 34)
        xc[:, :, lo - (r0 - 2):hi - (r0 - 2), 1:129] = x[b, :, :, lo:hi, :]
        # filter input, pixel-partition: xt[pc, t, r, c] = x[b, c, t, clip(r0-1+r), pc]
        rows = np.clip(np.arange(r0 - 1, r0 + 33), 0, H - 1)
        # x[b][:, :, rows, :] has shape (c, t, 34, w); -> (w, t, 34, c)
        xt = np.ascontiguousarray(x[b][:, :, rows, :].transpose(3, 1, 2, 0))
        # conv2 zero-pad mask for the y halo rows (grid rows 1 and 34)
        ymask = np.ones((DIM, 2), dtype=np.float32)
        if s == 0:
            ymask[:, 0] = 0.0
        if s == 3:
            ymask[:, 1] = 0.0
        in_maps.append({
            "xc": xc, "xt": xt, "w1t": w1t, "w2t": w2t,
            "b1c": b1c, "b1s": b1s, "b2c": b2c, "ymask": ymask, "emask": emask,
            "efold": efold, "emA": emA, "emB": emB, "ident": ident,
        })
    return in_maps


def kernel(x, w1, b1, w2, b2):
    from concourse.bass_utils import run_bass_kernel_spmd

    nc = _get_program()
    in_maps = _host_prep(x, w1, b1, w2, b2)
    res = run_bass_kernel_spmd(nc, in_maps, list(range(NCORES)))
    out = np.zeros((2, DIM, H, W), dtype=np.float32)
    for core in range(NCORES):
        b, s = divmod(core, 4)
        out[b, :, s * SLAB:(s + 1) * SLAB, :] = res.results[core]["out"]
    return out



# revision 19
# speedup vs baseline: 1.6773x; 1.6773x over previous
"""Trainium2 Bass kernel for nn_DynamicFiltering.

Computation (per batch b):
  xf = frames of x                     (t, c, h, w)
  y  = LeakyReLU(conv2d(xf, w1, b1), 0.2)
  ker = conv2d(y, w2, b2)              (t, 9, h, w)
  ker = ker - mean_k(ker) + 1/45       (per-pixel kernel over K = t*3*3 = 45)
  out[c,h,w] = sum_{t,k1,k2} x_edge[c,t,h+k1-1,w+k2-1] * ker[t,k1,k2][h,w]

Sharding: 8 cores = 2 batches x 4 H-slabs of 32 rows.

Per-core device program (v2, bf16):
  - conv1/conv2 as bf16 matmuls with K=128 tap pairing: the rhs tiles hold
    the image in partitions 0:64 and a one-row-shifted copy in 64:128, so
    taps (0,dj)+(1,dj) share one matmul and (2,dj) runs on the top half.
    6 matmuls per 4-row chunk instead of 9.
  - LeakyReLU fused into a single scalar-engine Lrelu activation.
  - per-frame kernel transpose to pixel-partition layout via one XBAR DMA
    transpose (16x128 tiles) + one DVE strided copy into [q, tap, r] form.
  - dynamic filtering on DVE in bf16 with every operand innermost-packed
    (xt is [q, c, r], kernel broadcast over c with innermost r) so the
    16-bit 2x mode applies; bf16 accumulators, one per dj column shift.
  - dj merge pre-transpose via DMA partition shifts; 16 f32r PE transposes;
    DMA out.
"""

import numpy as np

DIM = 64
T = 5
H = 128
W = 128
SLAB = 32          # output rows per core
NCORES = 8

_PROGRAM_CACHE = {}


def _build_program(debug=False):
    import concourse.bacc as bacc
    import concourse.mybir as mybir
    from concourse.tile import TileContext

    f32 = mybir.dt.float32
    f32r = mybir.dt.float32r
    bf16 = mybir.dt.bfloat16
    Act = mybir.ActivationFunctionType
    Alu = mybir.AluOpType

    nc = bacc.Bacc("TRN2", debug=False)

    xc_d = nc.dram_tensor("xc", [DIM, T, 37, 130], bf16, kind="ExternalInput").ap()
    xt_d = nc.dram_tensor("xt", [W, T, DIM, 34], bf16, kind="ExternalInput").ap()
    w1p_d = nc.dram_tensor("w1p", [128, 3, DIM], bf16, kind="ExternalInput").ap()
    w1s_d = nc.dram_tensor("w1s", [64, 3, DIM], bf16, kind="ExternalInput").ap()
    w2p_d = nc.dram_tensor("w2p", [128, 3, 9], bf16, kind="ExternalInput").ap()
    w2s_d = nc.dram_tensor("w2s", [64, 3, 9], bf16, kind="ExternalInput").ap()
    b1_d = nc.dram_tensor("b1c", [DIM, 1], f32, kind="ExternalInput").ap()
    b2_d = nc.dram_tensor("b2c", [9, 1], f32, kind="ExternalInput").ap()
    ym_d = nc.dram_tensor("ymask", [128, 2], f32, kind="ExternalInput").ap()
    em_d = nc.dram_tensor("emask", [W, 1], f32, kind="ExternalInput").ap()
    ef_d = nc.dram_tensor("efold", [W, 1], f32, kind="ExternalInput").ap()
    ea_d = nc.dram_tensor("emA", [W, 1], f32, kind="ExternalInput").ap()
    eb_d = nc.dram_tensor("emB", [W, 1], f32, kind="ExternalInput").ap()
    id_d = nc.dram_tensor("ident", [128, 128], f32, kind="ExternalInput").ap()
    out_d = nc.dram_tensor("out", [DIM, SLAB, W], f32, kind="ExternalOutput").ap()
    if debug:
        dbg_y = nc.dram_tensor("dbg_y", [128, 36, 130], bf16,
                               kind="ExternalOutput").ap()
        dbg_ker = nc.dram_tensor("dbg_ker", [16, SLAB, W], bf16,
                                 kind="ExternalOutput").ap()
        dbg_kta = nc.dram_tensor("dbg_kta", [W, SLAB, 16], bf16,
                                 kind="ExternalOutput").ap()
        dbg_kt2 = nc.dram_tensor("dbg_kt2", [W, T, 16, SLAB], bf16,
                                 kind="ExternalOutput").ap()

    with TileContext(nc) as tc:
        with (
            tc.tile_pool(name="consts", bufs=1) as cpool,
            tc.tile_pool(name="xcp", bufs=2) as xcp,
            tc.tile_pool(name="xtp", bufs=2) as xtp,
            tc.tile_pool(name="yp", bufs=2) as yp,
            tc.tile_pool(name="kerp", bufs=2) as kerp,
            tc.tile_pool(name="ktap", bufs=2) as ktap,
            tc.tile_pool(name="ktp", bufs=1) as ktp,
            tc.tile_pool(name="accp", bufs=1) as accp,
            tc.tile_pool(name="stage", bufs=3) as stp,
            tc.tile_pool(name="obp", bufs=3) as obp,
        ):
            w1p_sb = cpool.tile([128, 3, DIM], bf16)
            nc.sync.dma_start(out=w1p_sb, in_=w1p_d)
            w1s_sb = cpool.tile([128, 3, DIM], bf16)
            nc.sync.dma_start(out=w1s_sb[64:128], in_=w1s_d)
            w2p_sb = cpool.tile([128, 3, 9], bf16)
            nc.sync.dma_start(out=w2p_sb, in_=w2p_d)
            w2s_sb = cpool.tile([128, 3, 9], bf16)
            nc.sync.dma_start(out=w2s_sb[64:128], in_=w2s_d)
            b1_sb = cpool.tile([DIM, 1], f32)
            nc.sync.dma_start(out=b1_sb, in_=b1_d)
            b2_sb = cpool.tile([9, 1], f32)
            nc.sync.dma_start(out=b2_sb, in_=b2_d)
            ym_sb = cpool.tile([128, 2], f32)
            nc.sync.dma_start(out=ym_sb, in_=ym_d)
            em_sb = cpool.tile([W, 1], f32)
            nc.sync.dma_start(out=em_sb, in_=em_d)
            ef_sb = cpool.tile([W, 1], f32)
            nc.sync.dma_start(out=ef_sb, in_=ef_d)
            ea_sb = cpool.tile([W, 1], f32)
            nc.sync.dma_start(out=ea_sb, in_=ea_d)
            eb_sb = cpool.tile([W, 1], f32)
            nc.sync.dma_start(out=eb_sb, in_=eb_d)
            id_sb = cpool.tile([128, 128], f32)
            nc.sync.dma_start(out=id_sb, in_=id_d)
            al_sb = cpool.tile([DIM, 1], f32)
            nc.vector.memset(al_sb, 0.2)

            # per-pixel kernels, [q, frame, tap16, r] bf16 (taps 9..15 unused)
            kt2 = ktp.tile([W, T, 16, SLAB], bf16)
            kt_p1 = ktp.tile([W, T, 16, SLAB], bf16)
            kt_m1 = ktp.tile([W, T, 16, SLAB], bf16)
            nc.vector.memset(kt_p1[96:128], 0.0)
            nc.vector.memset(kt_m1[0:32], 0.0)

            # bf16 accumulators, one per dj; [q, c, r]
            accs = [accp.tile([W, DIM, SLAB], bf16, name=f"acc{dj}")
                    for dj in range(3)]
            ksrc = [kt_p1, kt2, kt_m1]
            u_sb = accp.tile([W, DIM, 34], bf16)

            def emit_conv1(f, ps1p):
                xp = xcp.tile([128, 36, 130], bf16, tag="xp")
                nc.sync.dma_start(out=xp[0:64], in_=xc_d[:, f, 0:36])
                nc.sync.dma_start(out=xp[64:128], in_=xc_d[:, f, 1:37])
                xt_f = xtp.tile([W, DIM, 34], bf16, tag="xt")
                nc.sync.dma_start(out=xt_f, in_=xt_d[:, f])

                y2 = yp.tile([128, 36, 130], bf16, tag="y2")
                nc.gpsimd.memset(y2[:, :, 0:1], 0.0)
                nc.gpsimd.memset(y2[:, :, 129:130], 0.0)
                if f == 0:
                    nc.gpsimd.tensor_copy(u_sb, xt_f)
                else:
                    nc.gpsimd.tensor_tensor(u_sb, u_sb, xt_f, Alu.add)

                for rc in range(9):
                    g0 = 1 + 4 * rc
                    nr = 4 if rc < 8 else 2
                    ps = ps1p.tile([DIM, 4, W], f32, tag="ps1")
                    for i, dj in enumerate(range(3)):
                        nc.tensor.matmul(
                            ps[:, :nr, :],
                            lhsT=w1p_sb[:, dj, :],
                            rhs=xp[:, g0 - 1:g0 - 1 + nr, dj:dj + W],
                            start=(i == 0),
                            stop=False,
                        )
                    for i, dj in enumerate(range(3)):
                        nc.tensor.matmul(
                            ps[:, :nr, :],
                            lhsT=w1s_sb[64:128, dj, :],
                            rhs=xp[64:128, g0:g0 + nr, dj:dj + W],
                            start=False,
                            stop=(i == 2),
                        )
                    nc.scalar.activation(y2[0:64, g0:g0 + nr, 1:129],
                                         ps[:, :nr], Act.Prelu,
                                         bias=b1_sb, scale=1.0, alpha=al_sb)
                # conv2 zero-pads rows outside the image: scale the y rows
                # that fall outside (mask is 0 there for edge slabs)
                nc.scalar.activation(y2[0:64, 1:2, 1:129], y2[0:64, 1:2, 1:129],
                                     Act.Copy, scale=ym_sb[0:64, 0:1])
                nc.scalar.activation(y2[0:64, 34:35, 1:129],
                                     y2[0:64, 34:35, 1:129],
                                     Act.Copy, scale=ym_sb[0:64, 1:2])
                # row-shifted second half for conv2 tap pairing
                nc.sync.dma_start(out=y2[64:128, 0:35], in_=y2[0:64, 1:36])
                if debug and f == 0:
                    nc.sync.dma_start(out=dbg_y, in_=y2)
                return xp, xt_f, y2

            def emit_conv2(f, y2, ps2p):
                ker16 = kerp.tile([16, SLAB, W], bf16, tag="ker16")
                for rc in range(8):
                    c0 = 2 + 4 * rc
                    ps2 = ps2p.tile([9, 4, W], f32, tag="ps2")
                    for i, dj in enumerate(range(3)):
                        nc.tensor.matmul(
                            ps2,
                            lhsT=w2p_sb[:, dj, :],
                            rhs=y2[:, c0 - 1:c0 + 3, dj:dj + W],
                            start=(i == 0),
                            stop=False,
                        )
                    for i, dj in enumerate(range(3)):
                        nc.tensor.matmul(
                            ps2,
                            lhsT=w2s_sb[64:128, dj, :],
                            rhs=y2[64:128, c0:c0 + 4, dj:dj + W],
                            start=False,
                            stop=(i == 2),
                        )
                    nc.scalar.activation(
                        ker16[0:9, 4 * rc:4 * rc + 4, :],
                        ps2, Act.Identity, bias=b2_sb, scale=1.0)

                # transpose (tap, r, q) -> (q, r, tap) via the DMA XBAR
                # (xbar block b of 128 cols lands at out[:, b, :]),
                # then repack to (q, tap, r) so filtering reads are
                # innermost-contiguous (enables the DVE 16-bit 2x mode)
                kt_a = ktap.tile([W, SLAB, 16], bf16, tag="kta")
                nc.sync.dma_start_transpose(
                    out=kt_a, in_=ker16.rearrange("k r q -> k (r q)"))
                nc.vector.tensor_copy(kt2[:, f],
                                      kt_a.rearrange("q r t -> q t r"))
                if debug and f == 0:
                    nc.sync.dma_start(out=dbg_ker, in_=ker16)
                    nc.sync.dma_start(out=dbg_kta, in_=kt_a)

                # fold W-edge replicate-pad terms into the dj=1 slot
                ktr = kt2[:, f, 0:9, :].rearrange("q (di dj) r -> q di dj r",
                                                  di=3, dj=3)
                nc.vector.tensor_tensor(ktr[0:1, :, 1, :], ktr[0:1, :, 1, :],
                                        ktr[0:1, :, 0, :], Alu.add)
                nc.vector.scalar_tensor_tensor(
                    out=ktr[96:128, :, 1, :],
                    in0=ktr[96:128, :, 2, :], scalar=em_sb[96:128, :],
                    in1=ktr[96:128, :, 1, :], op0=Alu.mult, op1=Alu.add)

                # partition-shifted kernel copies for the dj column shifts
                nc.sync.dma_start(out=kt_p1[0:127, f], in_=kt2[1:128, f])
                nc.sync.dma_start(out=kt_m1[1:128, f], in_=kt2[0:127, f])

            def emit_filter(f, xt_f):
                for dj in range(3):
                    prods = []
                    for di in range(3):
                        kb = ksrc[dj][:, f, 3 * di + dj, :].unsqueeze(1)\
                            .broadcast_to((W, DIM, SLAB))
                        prod = stp.tile([W, DIM, SLAB], bf16, tag="prod")
                        nc.vector.tensor_tensor(
                            prod, xt_f[:, :, di:di + SLAB], kb, Alu.mult)
                        prods.append(prod)
                    if f == 0:
                        nc.vector.tensor_tensor(accs[dj], prods[0], prods[1],
                                                Alu.add)
                    else:
                        nc.vector.tensor_tensor(accs[dj], accs[dj], prods[0],
                                                Alu.add)
                        nc.vector.tensor_tensor(accs[dj], accs[dj], prods[1],
                                                Alu.add)
                    nc.vector.tensor_tensor(accs[dj], accs[dj], prods[2],
                                            Alu.add)

            # software-pipelined frame loop: conv1(f) is emitted before
            # conv2(f-1) so the PE never waits on the y2 shift DMA
            with (
                tc.tile_pool(name="ps1", bufs=3, space="PSUM") as ps1p,
                tc.tile_pool(name="ps2", bufs=3, space="PSUM") as ps2p,
            ):
                prev = None
                for f in range(T + 1):
                    if f < T:
                        cur = emit_conv1(f, ps1p)
                    if prev is not None:
                        pf, (pxp, pxt, py2) = prev
                        emit_conv2(pf, py2, ps2p)
                        emit_filter(pf, pxt)
                    if f < T:
                        prev = (f, cur)

            if debug:
                nc.sync.dma_start(out=dbg_kt2, in_=kt2)

            # c = 1/45 - mean(ker); sum45 reads the folded kernel, so undo
            # the fold's double-count at the edge partitions.
            sum45 = ktp.tile([W, SLAB], f32)
            nc.vector.tensor_reduce(
                sum45, kt2[:, :, 0:9, :].rearrange("q f t r -> q r f t"),
                axis=mybir.AxisListType.XY, op=Alu.add)
            c_sb = ktp.tile([W, SLAB], f32)
            nc.vector.tensor_scalar(c_sb, sum45, -1.0 / 45.0, 1.0 / 45.0,
                                    Alu.mult, Alu.add)
            corr = ktp.tile([W, SLAB], f32)
            ktr_r = kt2[:, :, 0:9, :].rearrange("q f (di dj) r -> q r f di dj",
                                                di=3, dj=3)
            nc.vector.tensor_reduce(corr[0:32], ktr_r[0:32, :, :, :, 0],
                                    axis=mybir.AxisListType.XY, op=Alu.add)
            nc.vector.tensor_reduce(corr[96:128], ktr_r[96:128, :, :, :, 2],
                                    axis=mybir.AxisListType.XY, op=Alu.add)
            nc.vector.scalar_tensor_tensor(out=c_sb[0:32], in0=corr[0:32],
                                           scalar=ea_sb[0:32], in1=c_sb[0:32],
                                           op0=Alu.mult, op1=Alu.add)
            nc.vector.scalar_tensor_tensor(out=c_sb[96:128], in0=corr[96:128],
                                           scalar=eb_sb[96:128],
                                           in1=c_sb[96:128],
                                           op0=Alu.mult, op1=Alu.add)

            # S = 3-row vertical box of U (edge rows already clamped in xt)
            s_sb = accp.tile([W, DIM, SLAB], bf16)
            nc.vector.tensor_tensor(s_sb, u_sb[:, :, 0:SLAB],
                                    u_sb[:, :, 1:SLAB + 1], Alu.add)
            nc.vector.tensor_tensor(s_sb, s_sb, u_sb[:, :, 2:SLAB + 2],
                                    Alu.add)

            # shifted + edge-doubled variants of c, in bf16 for 2x filtering
            c_p1 = ktp.tile([W, SLAB], f32)
            c_m1 = ktp.tile([W, SLAB], f32)
            nc.vector.memset(c_p1[96:128], 0.0)
            nc.vector.memset(c_m1[0:32], 0.0)
            nc.sync.dma_start(out=c_p1[0:127], in_=c_sb[1:128])
            nc.sync.dma_start(out=c_m1[1:128], in_=c_sb[0:127])
            c_c = ktp.tile([W, SLAB], f32)
            nc.vector.tensor_scalar(c_c, c_sb, ef_sb, None, Alu.mult)
            cb_p1 = ktp.tile([W, SLAB], bf16)
            cb_c = ktp.tile([W, SLAB], bf16)
            cb_m1 = ktp.tile([W, SLAB], bf16)
            nc.vector.tensor_copy(cb_p1, c_p1)
            nc.vector.tensor_copy(cb_c, c_c)
            nc.vector.tensor_copy(cb_m1, c_m1)
            for dj, csrc in ((0, cb_p1), (1, cb_c), (2, cb_m1)):
                cbb = csrc.unsqueeze(1).broadcast_to((W, DIM, SLAB))
                prod = stp.tile([W, DIM, SLAB], bf16, tag="prod")
                nc.vector.tensor_tensor(prod, s_sb, cbb, Alu.mult)
                nc.vector.tensor_tensor(accs[dj], accs[dj], prod, Alu.add)

            # merge the dj accumulators pre-transpose via partition shifts:
            # A[q] = acc1[q] + acc0[q-1] + acc2[q+1]
            a0s = accp.tile([W, DIM, SLAB], bf16)
            a2s = accp.tile([W, DIM, SLAB], bf16)
            nc.vector.memset(a0s[0:32], 0.0)
            nc.vector.memset(a2s[96:128], 0.0)
            nc.sync.dma_start(out=a0s[1:128], in_=accs[0][0:127])
            nc.sync.dma_start(out=a2s[0:127], in_=accs[2][1:128])
            a_f32 = accp.tile([W, DIM, SLAB], f32)
            nc.vector.tensor_tensor(a_f32, accs[1], a0s, Alu.add)
            nc.vector.tensor_tensor(a_f32, a_f32, a2s, Alu.add)

            # transpose A (q, (c, r)) -> ((c, r), q) chunks and DMA out
            af = a_f32.rearrange("q c r -> q (c r)")
            with tc.tile_pool(name="pso", bufs=2, space="PSUM") as psop:
                for oc in range(16):
                    pso = psop.tile([128, 128], f32, tag="pso")
                    nc.tensor.transpose(pso, af[:, 128 * oc:128 * (oc + 1)],
                                        id_sb)
                    ob = obp.tile([128, 128], f32, tag="ob")
                    nc.scalar.activation(ob, pso, Act.Copy, scale=1.0)
                    nc.sync.dma_start(out=out_d[4 * oc:4 * oc + 4], in_=ob)

    return nc


def _get_program():
    if "nc" not in _PROGRAM_CACHE:
        nc = _build_program()
        nc.finalize()
        _PROGRAM_CACHE["nc"] = nc
    return _PROGRAM_CACHE["nc"]


def _get_program_debug():
    if "ncd" not in _PROGRAM_CACHE:
        nc = _build_program(debug=True)
        nc.finalize()
        _PROGRAM_CACHE["ncd"] = nc
    return _PROGRAM_CACHE["ncd"]


def _host_prep(x, w1, b1, w2, b2):
    """Build the 8 per-core input maps from full inputs."""
    import ml_dtypes
    bf16 = ml_dtypes.bfloat16

    x = np.asarray(x, dtype=np.float32)
    w1 = np.asarray(w1, dtype=np.float32)
    b1 = np.asarray(b1, dtype=np.float32)
    w2 = np.asarray(w2, dtype=np.float32)
    b2 = np.asarray(b2, dtype=np.float32)

    # paired conv weights: [pairs di=0,1 stacked on K, then di=2 single]
    # w1p[ci, dj, o] = w1[o, ci, 0, dj]; w1p[64+ci, dj, o] = w1[o, ci, 1, dj]
    w1p = np.concatenate([w1[:, :, 0, :].transpose(1, 2, 0),
                          w1[:, :, 1, :].transpose(1, 2, 0)], axis=0)
    w1s = np.ascontiguousarray(w1[:, :, 2, :].transpose(1, 2, 0))
    w2p = np.concatenate([w2[:, :, 0, :].transpose(1, 2, 0),
                          w2[:, :, 1, :].transpose(1, 2, 0)], axis=0)
    w2s = np.ascontiguousarray(w2[:, :, 2, :].transpose(1, 2, 0))

    b1c = np.ascontiguousarray(b1.reshape(DIM, 1))
    b2c = np.ascontiguousarray(b2.reshape(9, 1))
    ident = np.eye(128, dtype=np.float32)
    emask = np.zeros((W, 1), dtype=np.float32)
    emask[127, 0] = 1.0
    efold = np.ones((W, 1), dtype=np.float32)
    efold[0, 0] = 2.0
    efold[127, 0] = 2.0
    emA = np.zeros((W, 1), dtype=np.float32)
    emA[0, 0] = 1.0 / 45.0
    emB = np.zeros((W, 1), dtype=np.float32)
    emB[127, 0] = 1.0 / 45.0

    w1p = w1p.astype(bf16)
    w1s = w1s.astype(bf16)
    w2p = w2p.astype(bf16)
    w2s = w2s.astype(bf16)

    in_maps = []
    for core in range(NCORES):
        b, s = divmod(core, 4)
        r0 = s * SLAB
        # conv input: rows r0-2 .. r0+34 zero padded, cols -1..128 zero padded
        xc = np.zeros((DIM, T, 37, 130), dtype=np.float32)
        lo = max(0, r0 - 2)
        hi = min(H, r0 + 35)
        xc[:, :, lo - (r0 - 2):hi - (r0 - 2), 1:129] = x[b, :, :, lo:hi, :]
        # filter input, pixel-partition, innermost rows:
        # xt[q, t, c, j] = x[b, c, t, clip(r0-1+j), q]
        rows = np.clip(np.arange(r0 - 1, r0 + 33), 0, H - 1)
        xt = np.ascontiguousarray(x[b][:, :, rows, :].transpose(3, 1, 0, 2))
        # conv2 zero-pad mask for the y halo rows (y rows 1 and 34)
        ymask = np.ones((128, 2), dtype=np.float32)
        if s == 0:
            ymask[:, 0] = 0.0
        if s == 3:
            ymask[:, 1] = 0.0
        in_maps.append({
            "xc": xc.astype(bf16), "xt": xt.astype(bf16),
            "w1p": w1p, "w1s": w1s, "w2p": w2p, "w2s": w2s,
            "b1c": b1c, "b2c": b2c, "ymask": ymask, "emask": emask,
            "efold": efold, "emA": emA, "emB": emB, "ident": ident,
        })
    return in_maps


def kernel(x, w1, b1, w2, b2):
    from concourse.bass_utils import run_bass_kernel_spmd

    nc = _get_program()
    in_maps = _host_prep(x, w1, b1, w2, b2)
    res = run_bass_kernel_spmd(nc, in_maps, list(range(NCORES)))
    out = np.zeros((2, DIM, H, W), dtype=np.float32)
    for core in range(NCORES):
        b, s = divmod(core, 4)
        out[b, :, s * SLAB:(s + 1) * SLAB, :] = res.results[core]["out"]
    return out


# revision 31
# speedup vs baseline: 1.9803x; 1.1806x over previous
"""Trainium2 Bass kernel for nn_DynamicFiltering.

Computation (per batch b):
  xf = frames of x                     (t, c, h, w)
  y  = LeakyReLU(conv2d(xf, w1, b1), 0.2)
  ker = conv2d(y, w2, b2)              (t, 9, h, w)
  ker = ker - mean_k(ker) + 1/45       (per-pixel kernel over K = t*3*3 = 45)
  out[c,h,w] = sum_{t,k1,k2} x_edge[c,t,h+k1-1,w+k2-1] * ker[t,k1,k2][h,w]

Sharding: 8 cores = 2 batches x 4 H-slabs of 32 rows.

Per-core device program (v2, bf16):
  - conv1/conv2 as bf16 matmuls with K=128 tap pairing: the rhs tiles hold
    the image in partitions 0:64 and a one-row-shifted copy in 64:128, so
    taps (0,dj)+(1,dj) share one matmul and (2,dj) runs on the top half.
    6 matmuls per 4-row chunk instead of 9.
  - LeakyReLU fused into a single scalar-engine Lrelu activation.
  - per-frame kernel transpose to pixel-partition layout via one XBAR DMA
    transpose (16x128 tiles) + one DVE strided copy into [q, tap, r] form.
  - dynamic filtering on DVE in bf16 with every operand innermost-packed
    (xt is [q, c, r], kernel broadcast over c with innermost r) so the
    16-bit 2x mode applies; bf16 accumulators, one per dj column shift.
  - dj merge pre-transpose via DMA partition shifts; 16 f32r PE transposes;
    DMA out.
"""

import numpy as np

DIM = 64
T = 5
H = 128
W = 128
SLAB = 32          # output rows per core
NCORES = 8

_PROGRAM_CACHE = {}


def _build_program(debug=False):
    import concourse.bacc as bacc
    import concourse.mybir as mybir
    from concourse.tile import TileContext

    f32 = mybir.dt.float32
    f32r = mybir.dt.float32r
    bf16 = mybir.dt.bfloat16
    Act = mybir.ActivationFunctionType
    Alu = mybir.AluOpType

    nc = bacc.Bacc("TRN2", debug=False)

    xc_d = nc.dram_tensor("xc", [DIM, T, 37, 130], bf16, kind="ExternalInput").ap()
    xt_d = nc.dram_tensor("xt", [W, T, DIM, 34], bf16, kind="ExternalInput").ap()
    w1p_d = nc.dram_tensor("w1p", [128, 3, DIM], bf16, kind="ExternalInput").ap()
    w1q_d = nc.dram_tensor("w1q", [128, DIM], bf16, kind="ExternalInput").ap()
    w1s2_d = nc.dram_tensor("w1s2", [64, DIM], bf16, kind="ExternalInput").ap()
    w2p_d = nc.dram_tensor("w2p", [128, 3, 9], bf16, kind="ExternalInput").ap()
    w2s_d = nc.dram_tensor("w2s", [64, 3, 9], bf16, kind="ExternalInput").ap()
    b1_d = nc.dram_tensor("b1c", [DIM, 1], f32, kind="ExternalInput").ap()
    b2_d = nc.dram_tensor("b2c", [9, 1], f32, kind="ExternalInput").ap()
    ym_d = nc.dram_tensor("ymask", [128, 2], f32, kind="ExternalInput").ap()
    em_d = nc.dram_tensor("emask", [W, 1], f32, kind="ExternalInput").ap()
    ef_d = nc.dram_tensor("efold", [W, 1], f32, kind="ExternalInput").ap()
    ea_d = nc.dram_tensor("emA", [W, 1], f32, kind="ExternalInput").ap()
    eb_d = nc.dram_tensor("emB", [W, 1], f32, kind="ExternalInput").ap()
    # permutation matrices for the final fused transpose+shift matmuls
    idb_d = nc.dram_tensor("idb", [128, 128], bf16, kind="ExternalInput").ap()
    pdn_d = nc.dram_tensor("pdn", [128, 128], bf16, kind="ExternalInput").ap()
    pup_d = nc.dram_tensor("pup", [128, 128], bf16, kind="ExternalInput").ap()
    out_d = nc.dram_tensor("out", [DIM, SLAB, W], f32, kind="ExternalOutput").ap()
    if debug:
        dbg_y = nc.dram_tensor("dbg_y", [128, 36, 130], bf16,
                               kind="ExternalOutput").ap()
        dbg_ker = nc.dram_tensor("dbg_ker", [16, SLAB, W], bf16,
                                 kind="ExternalOutput").ap()
        dbg_kta = nc.dram_tensor("dbg_kta", [W, SLAB, 16], bf16,
                                 kind="ExternalOutput").ap()
        dbg_kt2 = nc.dram_tensor("dbg_kt2", [W, T, 16, SLAB], bf16,
                                 kind="ExternalOutput").ap()

    with TileContext(nc) as tc:
        with (
            tc.tile_pool(name="consts", bufs=1) as cpool,
            tc.tile_pool(name="xcp", bufs=2) as xcp,
            tc.tile_pool(name="xtp", bufs=2) as xtp,
            tc.tile_pool(name="yp", bufs=2) as yp,
            tc.tile_pool(name="kerp", bufs=2) as kerp,
            tc.tile_pool(name="ktap", bufs=2) as ktap,
            tc.tile_pool(name="ktp", bufs=1) as ktp,
            tc.tile_pool(name="accp", bufs=1) as accp,
            tc.tile_pool(name="stage", bufs=3) as stp,
            tc.tile_pool(name="obp", bufs=3) as obp,
        ):
            # consts are issued on the scalar-engine DGE so the sync DGE can
            # start streaming frame 0's inputs immediately
            w1p_sb = cpool.tile([128, 3, DIM], bf16)
            nc.scalar.dma_start(out=w1p_sb, in_=w1p_d)
            w1q_sb = cpool.tile([128, DIM], bf16)
            nc.scalar.dma_start(out=w1q_sb, in_=w1q_d)
            w1s2_sb = cpool.tile([64, DIM], bf16)
            nc.scalar.dma_start(out=w1s2_sb, in_=w1s2_d)
            w2p_sb = cpool.tile([128, 3, 9], bf16)
            nc.scalar.dma_start(out=w2p_sb, in_=w2p_d)
            w2s_sb = cpool.tile([128, 3, 9], bf16)
            nc.scalar.dma_start(out=w2s_sb[64:128], in_=w2s_d)
            b1_sb = cpool.tile([DIM, 1], f32)
            nc.scalar.dma_start(out=b1_sb, in_=b1_d)
            b2_sb = cpool.tile([9, 1], f32)
            nc.scalar.dma_start(out=b2_sb, in_=b2_d)
            ym_sb = cpool.tile([128, 2], f32)
            nc.scalar.dma_start(out=ym_sb, in_=ym_d)
            em_sb = cpool.tile([W, 1], f32)
            nc.scalar.dma_start(out=em_sb, in_=em_d)
            ef_sb = cpool.tile([W, 1], f32)
            nc.scalar.dma_start(out=ef_sb, in_=ef_d)
            ea_sb = cpool.tile([W, 1], f32)
            nc.scalar.dma_start(out=ea_sb, in_=ea_d)
            eb_sb = cpool.tile([W, 1], f32)
            nc.scalar.dma_start(out=eb_sb, in_=eb_d)
            idb_sb = cpool.tile([128, 128], bf16)
            nc.scalar.dma_start(out=idb_sb, in_=idb_d)
            pdn_sb = cpool.tile([128, 128], bf16)
            nc.scalar.dma_start(out=pdn_sb, in_=pdn_d)
            pup_sb = cpool.tile([128, 128], bf16)
            nc.scalar.dma_start(out=pup_sb, in_=pup_d)
            al_sb = cpool.tile([DIM, 1], f32)
            nc.vector.memset(al_sb, 0.2)

            # per-pixel kernels, [q, frame, tap16, r] bf16 (taps 9..15 unused)
            kt2 = ktp.tile([W, T, 16, SLAB], bf16)
            kt_p1 = ktp.tile([W, T, 16, SLAB], bf16)
            kt_m1 = ktp.tile([W, T, 16, SLAB], bf16)
            nc.gpsimd.memset(kt_p1[96:128], 0.0)
            nc.gpsimd.memset(kt_m1[0:32], 0.0)
            sum45 = ktp.tile([W, SLAB], f32)
            t45 = ktp.tile([W, SLAB], f32)

            # bf16 accumulators, one per dj; [q, c, r]
            accs = [accp.tile([W, DIM, SLAB], bf16, name=f"acc{dj}")
                    for dj in range(3)]
            ksrc = [kt_p1, kt2, kt_m1]
            u_sb = accp.tile([W, DIM, 34], bf16)

            def emit_conv1(f, ps1p):
                xp = xcp.tile([128, 36, 130], bf16, tag="xp")
                nc.sync.dma_start(out=xp[0:64], in_=xc_d[:, f, 0:36])
                nc.sync.dma_start(out=xp[64:128], in_=xc_d[:, f, 1:37])
                xq = xcp.tile([128, 36, 130], bf16, tag="xq")
                nc.sync.dma_start(out=xq[0:64], in_=xc_d[:, f, 1:37])
                nc.sync.dma_start(out=xq[64:128, :, 0:129],
                                  in_=xc_d[:, f, 1:37, 1:130])
                xt_f = xtp.tile([W, DIM, 34], bf16, tag="xt")
                nc.scalar.dma_start(out=xt_f, in_=xt_d[:, f])

                y2 = yp.tile([128, 36, 130], bf16, tag="y2")
                nc.gpsimd.memset(y2[:, :, 0:1], 0.0)
                nc.gpsimd.memset(y2[:, :, 129:130], 0.0)
                if f == 0:
                    nc.gpsimd.tensor_copy(u_sb, xt_f)
                else:
                    nc.gpsimd.tensor_tensor(u_sb, u_sb, xt_f, Alu.add)

                for rc in range(9):
                    g0 = 1 + 4 * rc
                    nr = 4 if rc < 8 else 2
                    ps = ps1p.tile([DIM, 4, W], f32, tag="ps1")
                    for i, dj in enumerate(range(3)):
                        nc.tensor.matmul(
                            ps[:, :nr, :],
                            lhsT=w1p_sb[:, dj, :],
                            rhs=xp[:, g0 - 1:g0 - 1 + nr, dj:dj + W],
                            start=(i == 0),
                            stop=False,
                        )
                    nc.tensor.matmul(
                        ps[:, :nr, :],
                        lhsT=w1q_sb,
                        rhs=xq[:, g0:g0 + nr, 0:W],
                        start=False,
                        stop=False,
                    )
                    nc.tensor.matmul(
                        ps[:, :nr, :],
                        lhsT=w1s2_sb,
                        rhs=xq[0:64, g0:g0 + nr, 2:2 + W],
                        start=False,
                        stop=True,
                    )
                    nc.scalar.activation(y2[0:64, g0:g0 + nr, 1:129],
                                         ps[:, :nr], Act.Prelu,
                                         bias=b1_sb, scale=1.0, alpha=al_sb)
                # conv2 zero-pads rows outside the image: scale the y rows
                # that fall outside (mask is 0 there for edge slabs)
                nc.scalar.activation(y2[0:64, 1:2, 1:129], y2[0:64, 1:2, 1:129],
                                     Act.Copy, scale=ym_sb[0:64, 0:1])
                nc.scalar.activation(y2[0:64, 34:35, 1:129],
                                     y2[0:64, 34:35, 1:129],
                                     Act.Copy, scale=ym_sb[0:64, 1:2])
                # row-shifted second half for conv2 tap pairing
                nc.sync.dma_start(out=y2[64:128, 0:35], in_=y2[0:64, 1:36])
                if debug and f == 0:
                    nc.sync.dma_start(out=dbg_y, in_=y2)
                return xp, xt_f, y2

            def emit_conv2(f, y2, ps2p):
                ker16 = kerp.tile([16, SLAB, W], bf16, tag="ker16")
                for rc in range(8):
                    c0 = 2 + 4 * rc
                    ps2 = ps2p.tile([9, 4, W], f32, tag="ps2")
                    for i, dj in enumerate(range(3)):
                        nc.tensor.matmul(
                            ps2,
                            lhsT=w2p_sb[:, dj, :],
                            rhs=y2[:, c0 - 1:c0 + 3, dj:dj + W],
                            start=(i == 0),
                            stop=False,
                        )
                    for i, dj in enumerate(range(3)):
                        nc.tensor.matmul(
                            ps2,
                            lhsT=w2s_sb[64:128, dj, :],
                            rhs=y2[64:128, c0:c0 + 4, dj:dj + W],
                            start=False,
                            stop=(i == 2),
                        )
                    nc.scalar.activation(
                        ker16[0:9, 4 * rc:4 * rc + 4, :],
                        ps2, Act.Identity, bias=b2_sb, scale=1.0)

                # transpose (tap, r, q) -> (q, r, tap) via the DMA XBAR
                # (xbar block b of 128 cols lands at out[:, b, :]),
                # then repack to (q, tap, r) so filtering reads are
                # innermost-contiguous (enables the DVE 16-bit 2x mode)
                kt_a = ktap.tile([W, SLAB, 16], bf16, tag="kta")
                nc.sync.dma_start_transpose(
                    out=kt_a, in_=ker16.rearrange("k r q -> k (r q)"))
                nc.vector.tensor_copy(kt2[:, f],
                                      kt_a.rearrange("q r t -> q t r"))
                if debug and f == 0:
                    nc.sync.dma_start(out=dbg_ker, in_=ker16)
                    nc.sync.dma_start(out=dbg_kta, in_=kt_a)

                # fold W-edge replicate-pad terms into the dj=1 slot
                ktr = kt2[:, f, 0:9, :].rearrange("q (di dj) r -> q di dj r",
                                                  di=3, dj=3)
                nc.vector.tensor_tensor(ktr[0:1, :, 1, :], ktr[0:1, :, 1, :],
                                        ktr[0:1, :, 0, :], Alu.add)
                nc.vector.scalar_tensor_tensor(
                    out=ktr[96:128, :, 1, :],
                    in0=ktr[96:128, :, 2, :], scalar=em_sb[96:128, :],
                    in1=ktr[96:128, :, 1, :], op0=Alu.mult, op1=Alu.add)

                # partition-shifted kernel copies for the dj column shifts
                nc.sync.dma_start(out=kt_p1[0:127, f], in_=kt2[1:128, f])
                nc.sync.dma_start(out=kt_m1[1:128, f], in_=kt2[0:127, f])

                # incremental sum of the 45 (folded) kernel taps
                t_out = sum45 if f == 0 else t45
                nc.vector.tensor_reduce(
                    t_out, kt2[:, f, 0:9, :].rearrange("q t r -> q r t"),
                    axis=mybir.AxisListType.X, op=Alu.add)
                if f > 0:
                    nc.vector.tensor_tensor(sum45, sum45, t45, Alu.add)

            def emit_filter(f, xt_f):
                for dj in range(3):
                    prods = []
                    for di in range(3):
                        kb = ksrc[dj][:, f, 3 * di + dj, :].unsqueeze(1)\
                            .broadcast_to((W, DIM, SLAB))
                        prod = stp.tile([W, DIM, SLAB], bf16, tag="prod")
                        nc.vector.tensor_tensor(
                            prod, xt_f[:, :, di:di + SLAB], kb, Alu.mult)
                        prods.append(prod)
                    if f == 0:
                        nc.vector.tensor_tensor(accs[dj], prods[0], prods[1],
                                                Alu.add)
                    else:
                        nc.vector.tensor_tensor(accs[dj], accs[dj], prods[0],
                                                Alu.add)
                        nc.vector.tensor_tensor(accs[dj], accs[dj], prods[1],
                                                Alu.add)
                    nc.vector.tensor_tensor(accs[dj], accs[dj], prods[2],
                                            Alu.add)

            # software-pipelined frame loop: conv1(f) is emitted before
            # conv2(f-1) so the PE never waits on the y2 shift DMA
            with (
                tc.tile_pool(name="ps1", bufs=3, space="PSUM") as ps1p,
                tc.tile_pool(name="ps2", bufs=3, space="PSUM") as ps2p,
            ):
                prev = None
                for f in range(T + 1):
                    if f < T:
                        cur = emit_conv1(f, ps1p)
                    if prev is not None:
                        pf, (pxp, pxt, py2) = prev
                        emit_conv2(pf, py2, ps2p)
                        emit_filter(pf, pxt)
                    if f < T:
                        prev = (f, cur)

            if debug:
                nc.sync.dma_start(out=dbg_kt2, in_=kt2)

            # c = 1/45 - mean(ker); sum45 reads the folded kernel, so undo
            # the fold's double-count at the edge partitions.
            c_sb = ktp.tile([W, SLAB], f32)
            nc.vector.tensor_scalar(c_sb, sum45, -1.0 / 45.0, 1.0 / 45.0,
                                    Alu.mult, Alu.add)
            corr = ktp.tile([W, SLAB], f32)
            ktr_r = kt2[:, :, 0:9, :].rearrange("q f (di dj) r -> q r f di dj",
                                                di=3, dj=3)
            nc.vector.tensor_reduce(corr[0:32], ktr_r[0:32, :, :, :, 0],
                                    axis=mybir.AxisListType.XY, op=Alu.add)
            nc.vector.tensor_reduce(corr[96:128], ktr_r[96:128, :, :, :, 2],
                                    axis=mybir.AxisListType.XY, op=Alu.add)
            nc.vector.scalar_tensor_tensor(out=c_sb[0:32], in0=corr[0:32],
                                           scalar=ea_sb[0:32], in1=c_sb[0:32],
                                           op0=Alu.mult, op1=Alu.add)
            nc.vector.scalar_tensor_tensor(out=c_sb[96:128], in0=corr[96:128],
                                           scalar=eb_sb[96:128],
                                           in1=c_sb[96:128],
                                           op0=Alu.mult, op1=Alu.add)

            # S = 3-row vertical box of U (edge rows already clamped in xt)
            s_sb = accp.tile([W, DIM, SLAB], bf16)
            nc.vector.tensor_tensor(s_sb, u_sb[:, :, 0:SLAB],
                                    u_sb[:, :, 1:SLAB + 1], Alu.add)
            nc.vector.tensor_tensor(s_sb, s_sb, u_sb[:, :, 2:SLAB + 2],
                                    Alu.add)

            # shifted + edge-doubled variants of c, in bf16 for 2x filtering
            c_p1 = ktp.tile([W, SLAB], f32)
            c_m1 = ktp.tile([W, SLAB], f32)
            nc.gpsimd.memset(c_p1[96:128], 0.0)
            nc.gpsimd.memset(c_m1[0:32], 0.0)
            nc.sync.dma_start(out=c_p1[0:127], in_=c_sb[1:128])
            nc.sync.dma_start(out=c_m1[1:128], in_=c_sb[0:127])
            c_c = ktp.tile([W, SLAB], f32)
            nc.vector.tensor_scalar(c_c, c_sb, ef_sb, None, Alu.mult)
            cb_p1 = ktp.tile([W, SLAB], bf16)
            cb_c = ktp.tile([W, SLAB], bf16)
            cb_m1 = ktp.tile([W, SLAB], bf16)
            nc.vector.tensor_copy(cb_p1, c_p1)
            nc.vector.tensor_copy(cb_c, c_c)
            nc.vector.tensor_copy(cb_m1, c_m1)
            for dj, csrc in ((0, cb_p1), (1, cb_c), (2, cb_m1)):
                cbb = csrc.unsqueeze(1).broadcast_to((W, DIM, SLAB))
                prod = stp.tile([W, DIM, SLAB], bf16, tag="prod")
                nc.vector.tensor_tensor(prod, s_sb, cbb, Alu.mult)
                nc.vector.tensor_tensor(accs[dj], accs[dj], prod, Alu.add)

            # fused transpose + dj merge via PSUM-accumulating permute
            # matmuls: out[m, p] = acc1[p, m] + acc0[p-1, m] + acc2[p+1, m]
            af0 = accs[0].rearrange("q c r -> q (c r)")
            af1 = accs[1].rearrange("q c r -> q (c r)")
            af2 = accs[2].rearrange("q c r -> q (c r)")
            with tc.tile_pool(name="pso", bufs=4, space="PSUM") as psop:
                for oc in range(16):
                    sl = slice(128 * oc, 128 * (oc + 1))
                    pso = psop.tile([128, 128], f32, tag="pso")
                    nc.tensor.matmul(pso, lhsT=af1[:, sl], rhs=idb_sb,
                                     start=True, stop=False)
                    nc.tensor.matmul(pso, lhsT=af0[:, sl], rhs=pdn_sb,
                                     start=False, stop=False)
                    nc.tensor.matmul(pso, lhsT=af2[:, sl], rhs=pup_sb,
                                     start=False, stop=True)
                    ob = obp.tile([128, 128], f32, tag="ob")
                    nc.scalar.activation(ob, pso, Act.Copy, scale=1.0)
                    eng = nc.sync if oc % 2 == 0 else nc.scalar
                    eng.dma_start(out=out_d[4 * oc:4 * oc + 4], in_=ob)

    return nc


def _get_program():
    if "nc" not in _PROGRAM_CACHE:
        nc = _build_program()
        nc.finalize()
        _PROGRAM_CACHE["nc"] = nc
    return _PROGRAM_CACHE["nc"]


def _get_program_debug():
    if "ncd" not in _PROGRAM_CACHE:
        nc = _build_program(debug=True)
        nc.finalize()
        _PROGRAM_CACHE["ncd"] = nc
    return _PROGRAM_CACHE["ncd"]


def _host_prep(x, w1, b1, w2, b2):
    """Build the 8 per-core input maps from full inputs."""
    import ml_dtypes
    bf16 = ml_dtypes.bfloat16

    x = np.asarray(x, dtype=np.float32)
    w1 = np.asarray(w1, dtype=np.float32)
    b1 = np.asarray(b1, dtype=np.float32)
    w2 = np.asarray(w2, dtype=np.float32)
    b2 = np.asarray(b2, dtype=np.float32)

    # paired conv weights: [pairs di=0,1 stacked on K, then di=2 single]
    # w1p[ci, dj, o] = w1[o, ci, 0, dj]; w1p[64+ci, dj, o] = w1[o, ci, 1, dj]
    w1p = np.concatenate([w1[:, :, 0, :].transpose(1, 2, 0),
                          w1[:, :, 1, :].transpose(1, 2, 0)], axis=0)
    # w1q pairs taps (2,0)+(2,1) on a column-shifted rhs; w1s2 is tap (2,2)
    w1q = np.concatenate([w1[:, :, 2, 0].T, w1[:, :, 2, 1].T], axis=0)
    w1s2 = np.ascontiguousarray(w1[:, :, 2, 2].T)
    w2p = np.concatenate([w2[:, :, 0, :].transpose(1, 2, 0),
                          w2[:, :, 1, :].transpose(1, 2, 0)], axis=0)
    w2s = np.ascontiguousarray(w2[:, :, 2, :].transpose(1, 2, 0))

    b1c = np.ascontiguousarray(b1.reshape(DIM, 1))
    b2c = np.ascontiguousarray(b2.reshape(9, 1))
    idb = np.eye(128, dtype=np.float32)
    pdn = np.zeros((128, 128), dtype=np.float32)   # pdn[k, p]=1 iff k==p-1
    pdn[np.arange(127), np.arange(1, 128)] = 1.0
    pup = np.zeros((128, 128), dtype=np.float32)   # pup[k, p]=1 iff k==p+1
    pup[np.arange(1, 128), np.arange(127)] = 1.0
    emask = np.zeros((W, 1), dtype=np.float32)
    emask[127, 0] = 1.0
    efold = np.ones((W, 1), dtype=np.float32)
    efold[0, 0] = 2.0
    efold[127, 0] = 2.0
    emA = np.zeros((W, 1), dtype=np.float32)
    emA[0, 0] = 1.0 / 45.0
    emB = np.zeros((W, 1), dtype=np.float32)
    emB[127, 0] = 1.0 / 45.0

    w1p = w1p.astype(bf16)
    w1q = w1q.astype(bf16)
    w1s2 = w1s2.astype(bf16)
    w2p = w2p.astype(bf16)
    w2s = w2s.astype(bf16)
    idb = idb.astype(bf16)
    pdn = pdn.astype(bf16)
    pup = pup.astype(bf16)

    in_maps = []
    for core in range(NCORES):
        b, s = divmod(core, 4)
        r0 = s * SLAB
        # conv input: rows r0-2 .. r0+34 zero padded, cols -1..128 zero padded
        xc = np.zeros((DIM, T, 37, 130), dtype=np.float32)
        lo = max(0, r0 - 2)
        hi = min(H, r0 + 35)
        xc[:, :, lo - (r0 - 2):hi - (r0 - 2), 1:129] = x[b, :, :, lo:hi, :]
        # filter input, pixel-partition, innermost rows:
        # xt[q, t, c, j] = x[b, c, t, clip(r0-1+j), q]
        rows = np.clip(np.arange(r0 - 1, r0 + 33), 0, H - 1)
        xt = np.ascontiguousarray(x[b][:, :, rows, :].transpose(3, 1, 0, 2))
        # conv2 zero-pad mask for the y halo rows (y rows 1 and 34)
        ymask = np.ones((128, 2), dtype=np.float32)
        if s == 0:
            ymask[:, 0] = 0.0
        if s == 3:
            ymask[:, 1] = 0.0
        in_maps.append({
            "xc": xc.astype(bf16), "xt": xt.astype(bf16),
            "w1p": w1p, "w1q": w1q, "w1s2": w1s2, "w2p": w2p, "w2s": w2s,
            "b1c": b1c, "b2c": b2c, "ymask": ymask, "emask": emask,
            "efold": efold, "emA": emA, "emB": emB,
            "idb": idb, "pdn": pdn, "pup": pup,
        })
    return in_maps


def kernel(x, w1, b1, w2, b2):
    from concourse.bass_utils import run_bass_kernel_spmd

    nc = _get_program()
    in_maps = _host_prep(x, w1, b1, w2, b2)
    res = run_bass_kernel_spmd(nc, in_maps, list(range(NCORES)))
    out = np.zeros((2, DIM, H, W), dtype=np.float32)
    for core in range(NCORES):
        b, s = divmod(core, 4)
        out[b, :, s * SLAB:(s + 1) * SLAB, :] = res.results[core]["out"]
    return out
